# revision 1
# baseline (speedup 1.0000x reference)
"""CWT (GMW filterbank) Trainium2 kernel.

Computes Wx = ifft(Psih * fft(reflect_pad(x)))[..., N1:N1+L] for
x (32, 2048) f32, Psih (256, 4096) f32 -> out (32, 256, 2048) complex64.

Strategy (8 NeuronCores, data-parallel over batch, 4 rows/core):
  - Reflect padding is folded into the forward DFT matrix on the host
    (xp = Pad @ x is linear), so the device contracts over the 2048
    original samples directly. Forward DFT: bf16 matmuls with fp32 PSUM
    accumulation, freq-on-partitions; the lowest 128 bins additionally
    use hi/lo-split bf16 operands (3 terms) because their rounding error
    dominates the small high-scale output rows.
  - P = Psih (.) xh computed with one broadcast DVE multiply per
    (k-tile, component), output bf16.
  - Psih is analytic (zero for k >= 2048) and each scale is bandpass, so
    the inverse DFT contracts only over the k-tiles where the octave's
    filters are non-negligible (33 of 128 (octave, k-tile) pairs).
  - Mirror symmetry E[k, 4096-n] = conj(E[k, n]) pairs the right half of
    the output window with the left half: U/V/W/Z products are computed
    over n in [1024, 2048) only (halves inverse-DFT matmuls and E-matrix
    traffic); right-half outputs are assembled on DVE with reversed
    (negative-stride) access patterns, and the self-paired n=2048 column
    comes from a tiny (-1)^k projection.
  - DVE assembles re/im interleaved staging tiles (DVE beats ACT copies
    by ~2x here); outputs DMA out as (b, a, n, 2) f32, viewed as
    complex64 on the host.

DFT/IDFT matrices are embedded in the NEFF as Const tensors. Measured on
8 axon-tunneled trn2 cores: ~3.0e-3 global rel err (worst row 4.6e-3),
~130-215 us per invocation (loop-amortized estimate; the sustained x2048
number is power-throttled, single-shot sessions measure ~130-190 us).

Build notes hard-won in this environment:
  - Use bacc.Bacc() + nc.compile(): Bacc.generate_event_semaphores()
    legalizes multi-wait instructions; plain bass.Bass() programs fail
    walrus codegen with "Too many sync wait commands".
  - DVE tensor_tensor may read only ONE operand from PSUM (walrus
    NCC_IBVF027); hence the V/W SBUF scratch copies.
  - A single big out-DMA (partition-split AP) serializes on one HWDGE
    queue; four 32-partition DMAs spread across queues are faster.
"""

import numpy as np
import ml_dtypes

import concourse.bass as bass
import concourse.bacc as bacc
import concourse.mybir as mybir
import concourse.tile as tile
from concourse.bass_utils import run_bass_kernel_spmd

BF16 = ml_dtypes.bfloat16

B = 32          # batch
L = 2048        # signal length
UP = 4096       # padded length
N1 = 1024       # left pad (slice offset)
NA = 256        # scales
NV = 32         # voices/octave
NO = 8          # octaves
KF = 2048       # used frequency bins (0..2047; Psih==0 at k=0 and k>=2048)
NC = 8          # cores
BPC = B // NC   # batch rows per core (4)
KT = KF // 128  # k tiles (16)
MT = L // 128   # time tiles for forward (16)
NTILE = 512     # output columns per matmul
NT = L // NTILE  # n tiles (4)

_CACHE = {}


def _host_constants(Psih):
    """Build DFT/IDFT constant tensors + per-octave band table."""
    # folded forward DFT: xh[b, k] = sum_m x[b, m] * Ff[m, k]
    idx = np.concatenate([
        np.arange(N1, 0, -1),            # left reflect pad
        np.arange(0, L),                 # body
        np.arange(L - 2, L - 2 - (UP - L - N1), -1),  # right reflect pad
    ])
    n_ar = np.arange(UP)[:, None]
    k_ar = np.arange(KF)[None, :]
    F = np.exp(-2j * np.pi * n_ar * k_ar / UP)
    Ff = np.zeros((L, KF), dtype=np.complex128)
    np.add.at(Ff, idx, F)
    # device layout: (ft, m_in 128, mt, ri, k 128) bf16
    Ffc = Ff.real.reshape(MT, 128, KT, 128)
    Ffs = Ff.imag.reshape(MT, 128, KT, 128)
    ff_dev = np.empty((KT, 128, MT, 2, 128), dtype=BF16)
    ff_dev[:, :, :, 0, :] = Ffc.transpose(2, 1, 0, 3).astype(BF16)
    ff_dev[:, :, :, 1, :] = Ffs.transpose(2, 1, 0, 3).astype(BF16)

    # inverse DFT, output slice n in [N1, N1+L): E[k, n] = exp(2i pi k (N1+n)/UP)/UP
    kk = np.arange(KF)[:, None]
    nn = np.arange(N1, N1 + L)[None, :]
    E = np.exp(2j * np.pi * kk * nn / UP) / UP
    # device layout: (nt, kt, k_in 128, ri, n 512) bf16
    Er = E.real.reshape(KT, 128, NT, NTILE)
    Ei = E.imag.reshape(KT, 128, NT, NTILE)
    e_dev = np.empty((NT, KT, 128, 2, NTILE), dtype=BF16)
    e_dev[:, :, :, 0, :] = Er.transpose(2, 0, 1, 3).astype(BF16)
    e_dev[:, :, :, 1, :] = Ei.transpose(2, 0, 1, 3).astype(BF16)

    # lo-residual planes of Ff for k-tile 0 (hi/lo bf16 forward for the
    # lowest 128 bins, where bf16 rounding dominates small-row error)
    Fr0 = Ff.real[:, :128]; Fi0 = Ff.imag[:, :128]
    fflo_dev = np.empty((128, MT, 2, 128), dtype=BF16)
    Frh = Fr0.astype(BF16).astype(np.float64)
    Fih = Fi0.astype(BF16).astype(np.float64)
    fflo_dev[:, :, 0, :] = (Fr0 - Frh).astype(BF16).reshape(MT, 128, 128).transpose(1, 0, 2)
    fflo_dev[:, :, 1, :] = (Fi0 - Fih).astype(BF16).reshape(MT, 128, 128).transpose(1, 0, 2)

    # center column n=2048: E[k,2048] = (-1)^k / UP (imag part exactly 0)
    epm_dev = np.empty((KT, 128, 1), dtype=BF16)
    epm_dev[:, :, 0] = ((-1.0) ** (np.arange(KF) % 2) / UP).reshape(KT, 128).astype(BF16)

    # PsihT device layout: (k_in 128, kt, a 256) f32
    psiht = np.ascontiguousarray(
        Psih[:, :KF].T.reshape(KT, 128, NA).transpose(1, 0, 2)
    ).astype(np.float32)

    # per-octave k-tile bands (threshold relative to Psih peak value 2.0)
    bands = []
    for o in range(NO):
        sub = Psih[NV * o:NV * (o + 1), :KF]
        ks = np.nonzero((sub > 1e-4 * 2.0).any(axis=0))[0]
        bands.append((int(ks.min()) // 128, int(ks.max()) // 128 + 1))

    return ff_dev, e_dev, epm_dev, fflo_dev, psiht, bands


def _build_program(ff_dev, e_dev, epm_dev, fflo_dev, bands, reps=1, variant="full"):
    f32 = mybir.dt.float32
    bf16 = mybir.dt.bfloat16

    nc = bacc.Bacc()
    x_in = nc.dram_tensor("x", [BPC, L], f32, kind="ExternalInput")
    psih_in = nc.dram_tensor("psiht", [128, KT, NA], f32, kind="ExternalInput")
    out_t = nc.dram_tensor("out", [BPC, NA, L, 2], f32, kind="ExternalOutput")

    ff_c = nc.inline_tensor(ff_dev, name="ffconst")
    if "nomir" in variant:
        e_c = nc.inline_tensor(e_dev, name="econst")
    else:
        e_c = nc.inline_tensor(np.ascontiguousarray(e_dev[:2]), name="econst")
    epm_c = nc.inline_tensor(epm_dev, name="epmconst")
    fflo_c = nc.inline_tensor(fflo_dev, name="ffloconst")
    id_c = nc.inline_tensor(np.eye(BPC, dtype=np.float32), name="idconst")

    with tile.TileContext(nc) as tc:
        with (
            tc.tile_pool(name="persist", bufs=1) as persist,
            tc.tile_pool(name="pfix", bufs=1) as pfix,
            tc.tile_pool(name="ffp", bufs=4 if "ff4" in variant else 2) as ffp,
            tc.tile_pool(name="ep", bufs=20 if "nomir" in variant else 1) as ep,
            tc.tile_pool(name="stg", bufs=3) as stgp,
            tc.tile_pool(name="ps_f", bufs=2, space="PSUM") as ps_f,
            tc.tile_pool(name="ps_m", bufs=2 if "nomir" not in variant else 3, space="PSUM") as ps_m,
        ):
            # ---- prologue: load x, psih, identity; transpose x ----
            x_sb = persist.tile([BPC, L], f32, tag="x")
            nc.sync.dma_start(out=x_sb, in_=x_in[:])
            psih_sb = persist.tile([128, KT, NA], f32, tag="psih")
            nc.sync.dma_start(out=psih_sb, in_=psih_in[:])
            # funnel: absorb the psih DMA wait into DVE's clock so later DVE
            # ops reading psih_sb carry only their same-engine wait (walrus
            # allows a single sync wait on TensorTensor).
            scratch = persist.tile([1, 4], f32, tag="scratch")
            nc.vector.tensor_copy(out=scratch[0:1, 0:1], in_=psih_sb[0:1, 0, 0:1])
            id_sb = persist.tile([BPC, BPC], f32, tag="id")
            nc.sync.dma_start(out=id_sb, in_=id_c[:])

            # dummy transpose consumes the identity-DMA dependency on PE, so
            # the real transposes carry a single sync wait (walrus limit on
            # the transpose-mode LDWEIGHTS struct).
            mir = "nomir" not in variant

            def small_psum():
                if mir:
                    t = ps_m.tile([128, 2, NTILE], f32, tag="uv", name="sp")
                    return t[:, 0, 0:BPC]
                return ps_f.tile([128, BPC], f32, tag="fwd", name="spf")

            dummy = small_psum()
            nc.tensor.transpose(dummy[0:BPC, 0:BPC], id_sb, id_sb)

            xT = []
            for mt in range(MT):
                pst = small_psum()
                nc.tensor.transpose(
                    pst, x_sb[0:BPC, 128 * mt:128 * (mt + 1)], id_sb
                )
                xt = persist.tile([128, BPC], bf16, tag=f"xT{mt}")
                nc.vector.tensor_copy(out=xt, in_=pst)
                xl = persist.tile([128, BPC], bf16, tag=f"xL{mt}")
                nc.vector.tensor_sub(xl, pst, xt)
                xT.append((xt, xl))

            pre_e = None
            if "noe" in variant:
                pre_e = {}
                for kt in range(KT):
                    et = persist.tile([128, 2, NTILE], bf16, tag=f"pe{kt}")
                    nc.sync.dma_start(out=et, in_=e_c[0, kt])
                    pre_e[kt] = et

            def body():
                _emit_body(
                    nc, tc, bands, ff_c, e_c, epm_c, fflo_c, out_t,
                    persist, pfix, ffp, ep, stgp, ps_f, ps_m,
                    psih_sb, xT, f32, bf16, variant, pre_e,
                )

            if reps == 1:
                body()
            else:
                with tc.For_i(0, reps, 1):
                    body()
    nc.compile()
    return nc


def _emit_body(nc, tc, bands, ff_c, e_c, epm_c, fflo_c, out_t, persist, pfix,
       ffp, ep, stgp, ps_f, ps_m, psih_sb, xT, f32, bf16, variant="full",
       pre_e=None):
    skip_fwd = "nofwd" in variant
    skip_out = "noout" in variant
    skip_mm = "nomm" in variant
    # ---- forward DFT + P generation, high freq tiles first ----
    P_re, P_im, P_imn = {}, {}, {}
    for ft in reversed(range(KT)):
        psih_ap = (
            psih_sb[:, ft, :]
            .rearrange("p (o a) -> p o a", o=NO)[:, :, None, :]
            .to_broadcast((128, NO, BPC, NV))
        )
        if skip_fwd:
            stub = ((P_re, 0), (P_im, 1), (P_imn, 2)) if "nomir" in variant \
                else ((P_re, 0), (P_im, 1))
            for dst, comp in stub:
                pt = pfix.tile([128, NO * BPC * NV], bf16, tag=f"P{comp}_{ft}")
                nc.vector.tensor_copy(
                    out=pt.rearrange("p (o b a) -> p o b a", o=NO, b=BPC),
                    in_=psih_ap,
                )
                dst[ft] = pt
            continue
        ffri = ffp.tile([128, MT, 2, 128], bf16, tag="ff")
        nc.sync.dma_start(out=ffri, in_=ff_c[ft])
        if "nomir" in variant:
            psr = ps_f.tile([128, BPC], f32, tag="fwd")
            psi = ps_f.tile([128, BPC], f32, tag="fwd")
        else:
            fb = ps_m.tile([128, 2, NTILE], f32, tag="uv")
            psr = fb[:, 0, 0:BPC]
            psi = fb[:, 1, 0:BPC]
        hilo = ft == 0
        if hilo:
            fflo = ffp.tile([128, MT, 2, 128], bf16, tag="fflo", bufs=1)
            nc.sync.dma_start(out=fflo, in_=fflo_c[:])
        for ri, ps in ((0, psr), (1, psi)):
            for mt in range(MT):
                nc.tensor.matmul(
                    ps, ffri[:, mt, ri, :], xT[mt][0],
                    start=(mt == 0), stop=(mt == MT - 1 and not hilo),
                )
            if hilo:
                for mt in range(MT):
                    nc.tensor.matmul(ps, fflo[:, mt, ri, :], xT[mt][0],
                                     start=False, stop=False)
                for mt in range(MT):
                    nc.tensor.matmul(ps, ffri[:, mt, ri, :], xT[mt][1],
                                     start=False, stop=(mt == MT - 1))
        xh = persist.tile([128, 3, BPC], f32, tag=f"xh{ft}")
        nc.vector.tensor_copy(out=xh[:, 0, :], in_=psr)
        nc.vector.tensor_copy(out=xh[:, 1, :], in_=psi)
        if "nomir" in variant:
            nc.vector.tensor_scalar_mul(xh[:, 2, :], psi, -1.0)
            comps = ((P_re, 0), (P_im, 1), (P_imn, 2))
        else:
            comps = ((P_re, 0), (P_im, 1))

        for dst, comp in comps:
            pt = pfix.tile([128, NO * BPC * NV], bf16, tag=f"P{comp}_{ft}")
            xh_ap = (
                xh[:, comp, :][:, None, :, None]
                .to_broadcast((128, NO, BPC, NV))
            )
            eng = nc.gpsimd if ("pgsplit" in variant and comp == 1) else nc.vector
            eng.tensor_tensor(
                pt.rearrange("p (o b a) -> p o b a", o=NO, b=BPC),
                psih_ap,
                xh_ap,
                mybir.AluOpType.mult,
            )
            dst[ft] = pt

    if "nomir" not in variant:
        _emit_main_mirror(nc, bands, e_c, epm_c, out_t, persist, ep, stgp,
                          ps_m, P_re, P_im, f32, bf16, skip_out, skip_mm,
                          variant)
        return

    # ---- banded inverse DFT + output ----
    for nt in range(NT):
        if pre_e is not None:
            etiles = pre_e
        else:
            etiles = {}
            for kt in range(KT):
                et = ep.tile([128, 2, NTILE], bf16, tag="e")
                nc.sync.dma_start(out=et, in_=e_c[nt, kt])
                etiles[kt] = et
        for o in range(NO):
            klo, khi = bands[o]
            kts = list(range(klo, khi))
            osl = slice(o * 128, (o + 1) * 128)
            if skip_mm:
                continue
            big = ps_m.tile([128, 2, NTILE], f32, tag="acc")
            psr = big[:, 0, :]
            psi = big[:, 1, :]
            if not skip_mm:
                for j, kt in enumerate(kts):
                    first, last = (j == 0), (j == len(kts) - 1)
                    er = etiles[kt][:, 0, :]
                    ei = etiles[kt][:, 1, :]
                    nc.tensor.matmul(
                        psr, P_re[kt][:, osl], er, start=first, stop=False
                    )
                    nc.tensor.matmul(
                        psr, P_imn[kt][:, osl], ei, start=False, stop=last
                    )
                    nc.tensor.matmul(
                        psi, P_re[kt][:, osl], ei, start=first, stop=False
                    )
                    nc.tensor.matmul(
                        psi, P_im[kt][:, osl], er, start=False, stop=last
                    )
            if skip_mm:
                continue
            stg = stgp.tile([128, NTILE, 2], f32, tag="stg")
            if "actcopy" not in variant:
                nc.vector.tensor_copy(out=stg[:, :, 0], in_=psr)
                nc.vector.tensor_copy(out=stg[:, :, 1], in_=psi)
            else:
                nc.scalar.copy(out=stg[:, :, 0], in_=psr)
                nc.scalar.copy(out=stg[:, :, 1], in_=psi)
            if not skip_out:
                if "mergeout" not in variant:
                    for bl in range(BPC):
                        nc.sync.dma_start(
                            out=out_t[
                                bl,
                                NV * o:NV * (o + 1),
                                NTILE * nt:NTILE * (nt + 1),
                                :,
                            ],
                            in_=stg[NV * bl:NV * (bl + 1), :, :],
                        )
                else:
                    nc.sync.dma_start(
                        out=out_t[
                            :,
                            NV * o:NV * (o + 1),
                            NTILE * nt:NTILE * (nt + 1),
                            :,
                        ],
                        in_=stg,
                    )


def _get_program(Psih, reps=1, variant="full"):
    key = f"prog{reps}_{variant}"
    if key not in _CACHE:
        if "consts" not in _CACHE:
            _CACHE["consts"] = _host_constants(np.asarray(Psih))
        ff_dev, e_dev, epm_dev, fflo_dev, psiht, bands = _CACHE["consts"]
        nc = _build_program(ff_dev, e_dev, epm_dev, fflo_dev, bands,
                            reps=reps, variant=variant)
        _CACHE[key] = (nc, psiht)
    return _CACHE[key]


def kernel(x, Psih=None, **_unused):
    x = np.ascontiguousarray(np.asarray(x), dtype=np.float32)
    if Psih is None:
        raise ValueError("Psih input required")
    nc, psiht = _get_program(Psih)
    in_maps = [
        {"x": np.ascontiguousarray(x[BPC * c:BPC * (c + 1)]), "psiht": psiht}
        for c in range(NC)
    ]
    res = run_bass_kernel_spmd(nc, in_maps, core_ids=list(range(NC)))
    out = np.concatenate([r["out"] for r in res.results], axis=0)
    return out.view(np.complex64)[..., 0]


def bench(x, Psih, iters=20, reps=1, variant="full"):
    """Run the kernel repeatedly on-device; returns (out_complex, times_ns).

    Builds the same shard_map executable as bass2jax.run_bass_via_pjrt but
    without donation, so the warm executable can be re-invoked with
    device-resident inputs. Wall time per call (minus dispatch overhead)
    upper-bounds HW exec time.
    """
    import time
    import jax
    from jax.sharding import Mesh, PartitionSpec
    from jax.experimental.shard_map import shard_map
    from concourse import bass2jax

    x = np.ascontiguousarray(np.asarray(x), dtype=np.float32)
    nc, psiht = _get_program(Psih, reps=reps, variant=variant)
    bass2jax.install_neuronx_cc_hook()

    part_name = nc.partition_id_tensor.name if nc.partition_id_tensor else None
    in_names, out_names, out_avals = [], [], []
    for alloc in nc.m.functions[0].allocations:
        if not isinstance(alloc, mybir.MemoryLocationSet):
            continue
        name = alloc.memorylocations[0].name
        if alloc.kind == "ExternalInput":
            if name != part_name:
                in_names.append(name)
        elif alloc.kind == "ExternalOutput":
            out_names.append(name)
            out_avals.append(
                jax.core.ShapedArray(
                    tuple(alloc.tensor_shape), mybir.dt.np(alloc.dtype)
                )
            )
    n_params = len(in_names)
    all_names = in_names + out_names
    if part_name is not None:
        all_names = all_names + [part_name]

    def _body(*args):
        operands = list(args)
        if part_name is not None:
            operands.append(bass2jax.partition_id_tensor())
        outs = bass2jax._bass_exec_p.bind(
            *operands,
            out_avals=tuple(out_avals),
            in_names=tuple(all_names),
            out_names=tuple(out_names),
            lowering_input_output_aliases=(),
            sim_require_finite=True,
            sim_require_nnan=True,
            nc=nc,
        )
        return tuple(outs)

    devices = jax.devices()[:NC]
    mesh = Mesh(np.asarray(devices), ("core",))
    nin = n_params + len(out_names)
    fn = jax.jit(
        shard_map(
            _body,
            mesh=mesh,
            in_specs=(PartitionSpec("core"),) * nin,
            out_specs=(PartitionSpec("core"),) * len(out_names),
            check_rep=False,
        ),
        keep_unused=True,
    )
    in_map = {"x": x, "psiht": np.concatenate([psiht] * NC, axis=0)}
    concat_in = [in_map[n] for n in in_names]
    concat_zeros = [
        np.zeros((NC * a.shape[0], *a.shape[1:]), a.dtype) for a in out_avals
    ]
    sharding = jax.sharding.NamedSharding(mesh, PartitionSpec("core"))
    args = [jax.device_put(a, sharding) for a in concat_in + concat_zeros]
    out_arrs = jax.block_until_ready(fn(*args))  # compile + first run
    times = []
    for _ in range(iters):
        t0 = time.perf_counter()
        out_arrs = jax.block_until_ready(fn(*args))
        times.append((time.perf_counter() - t0) * 1e9)
    out = np.asarray(out_arrs[0]).reshape(NC, BPC, NA, L, 2).reshape(B, NA, L, 2)
    return out.view(np.complex64)[..., 0], times


def _rev_ap(ap2d, last_col, count):
    """Columns [last_col, last_col-1, ..., last_col-count+1] of a [128, C] AP."""
    import concourse.bass as bass
    return bass.AP(
        ap2d.tensor,
        ap2d.offset + last_col * ap2d.ap[-1][0],
        [list(ap2d.ap[0]), [-ap2d.ap[-1][0], count]],
    )


def _emit_main_mirror(nc, bands, e_c, epm_c, out_t, persist, ep, stgp, ps_m,
                      P_re, P_im, f32, bf16, skip_out, skip_mm, variant):
    """Mirror-symmetric inverse DFT: E[k, 4096-n] = conj(E[k, n]) pairs the
    right half of the output window with the left half, so U=Pre@Er, V=Pim@Ei,
    W=Pre@Ei, Z=Pim@Er are computed over n in [1024, 2048) only:
      out_re[1024+c] = U-V,  out_im = W+Z          (left)
      out_re[4096-m] = U+V,  out_im = Z-W at col m (right, reversed)
    n=2048 comes from a tiny (-1)^k projection (ctr)."""
    # E tiles for both left n-tiles, high k first (octave 0 starts earliest)
    etiles = {}
    for kt in reversed(range(KT)):
        for lnt in range(2):
            et = ep.tile([128, 2, NTILE], bf16, tag=f"e{lnt}_{kt}")
            nc.sync.dma_start(out=et, in_=e_c[lnt, kt])
            etiles[(lnt, kt)] = et
    epm_sb = persist.tile([128, KT, 1], bf16, tag="epm")
    nc.sync.dma_start(out=epm_sb, in_=epm_c[:].rearrange("t p o -> p t o"))

    for o in range(NO):
        klo, khi = bands[o]
        kts = list(range(klo, khi))
        osl = slice(o * 128, (o + 1) * 128)
        quads = {}
        for lnt in range(2):
            if skip_mm:
                continue
            uv = ps_m.tile([128, 2, NTILE], f32, tag="uv")
            wz = ps_m.tile([128, 2, NTILE], f32, tag="wz")
            if True:
                for j, kt in enumerate(kts):
                    first, last = (j == 0), (j == len(kts) - 1)
                    er = etiles[(lnt, kt)][:, 0, :]
                    ei = etiles[(lnt, kt)][:, 1, :]
                    nc.tensor.matmul(uv[:, 0, :], P_re[kt][:, osl], er,
                                     start=first, stop=last)
                    nc.tensor.matmul(uv[:, 1, :], P_im[kt][:, osl], ei,
                                     start=first, stop=last)
                    nc.tensor.matmul(wz[:, 0, :], P_re[kt][:, osl], ei,
                                     start=first, stop=last)
                    nc.tensor.matmul(wz[:, 1, :], P_im[kt][:, osl], er,
                                     start=first, stop=last)
            # V and W copied to SBUF once; reused by left and mirrored
            # right combines (DVE may read only one PSUM operand per op)
            vw = stgp.tile([128, 2, NTILE], f32, tag="vw")
            nc.vector.tensor_copy(out=vw[:, 0, :], in_=uv[:, 1, :])
            nc.vector.tensor_copy(out=vw[:, 1, :], in_=wz[:, 0, :])
            quads[lnt] = (uv, wz, vw)
            # left output for this tile: re = U - V, im = Z + W
            stg = stgp.tile([128, NTILE, 2], f32, tag="stg")
            nc.vector.tensor_sub(stg[:, :, 0], uv[:, 0, :], vw[:, 0, :])
            nc.vector.tensor_add(stg[:, :, 1], wz[:, 1, :], vw[:, 1, :])
            if not skip_out:
                for bl in range(BPC):
                    nc.sync.dma_start(
                        out=out_t[bl, NV * o:NV * (o + 1),
                                  NTILE * lnt:NTILE * (lnt + 1), :],
                        in_=stg[NV * bl:NV * (bl + 1), :, :],
                    )
        if skip_mm:
            continue

        uv0, wz0, vw0 = quads[0]
        uv1, wz1, vw1 = quads[1]

        # right tile 1: n in [2560, 3072) -> mirror cols of left tile 0
        stg = stgp.tile([128, NTILE, 2], f32, tag="stg")
        nc.vector.tensor_add(stg[:, 1:NTILE, 0],
                             _rev_ap(uv0[:, 0, :], NTILE - 1, NTILE - 1),
                             _rev_ap(vw0[:, 0, :], NTILE - 1, NTILE - 1))
        nc.vector.tensor_sub(stg[:, 1:NTILE, 1],
                             _rev_ap(wz0[:, 1, :], NTILE - 1, NTILE - 1),
                             _rev_ap(vw0[:, 1, :], NTILE - 1, NTILE - 1))
        nc.vector.tensor_add(stg[:, 0:1, 0], uv1[:, 0, 0:1], vw1[:, 0, 0:1])
        nc.vector.tensor_sub(stg[:, 0:1, 1], wz1[:, 1, 0:1], vw1[:, 1, 0:1])
        if not skip_out:
            for bl in range(BPC):
                nc.sync.dma_start(
                    out=out_t[bl, NV * o:NV * (o + 1), 3 * NTILE:4 * NTILE, :],
                    in_=stg[NV * bl:NV * (bl + 1), :, :],
                )

        # ctr: n = 2048 projection with (-1)^k / UP
        ctr = ps_m.tile([128, 2, NTILE], f32, tag="uv")
        for j, kt in enumerate(kts):
            first, last = (j == 0), (j == len(kts) - 1)
            nc.tensor.matmul(ctr[:, 0, 0:1], P_re[kt][:, osl],
                             epm_sb[:, kt, :], start=first, stop=last)
            nc.tensor.matmul(ctr[:, 1, 0:1], P_im[kt][:, osl],
                             epm_sb[:, kt, :], start=first, stop=last)

        # right tile 0: n in [2048, 2560): col0 = ctr, rest mirrors left tile 1
        stg = stgp.tile([128, NTILE, 2], f32, tag="stg")
        nc.vector.tensor_add(stg[:, 1:NTILE, 0],
                             _rev_ap(uv1[:, 0, :], NTILE - 1, NTILE - 1),
                             _rev_ap(vw1[:, 0, :], NTILE - 1, NTILE - 1))
        nc.vector.tensor_sub(stg[:, 1:NTILE, 1],
                             _rev_ap(wz1[:, 1, :], NTILE - 1, NTILE - 1),
                             _rev_ap(vw1[:, 1, :], NTILE - 1, NTILE - 1))
        nc.vector.tensor_copy(out=stg[:, 0:1, 0], in_=ctr[:, 0, 0:1])
        nc.vector.tensor_copy(out=stg[:, 0:1, 1], in_=ctr[:, 1, 0:1])
        if not skip_out:
            for bl in range(BPC):
                nc.sync.dma_start(
                    out=out_t[bl, NV * o:NV * (o + 1), 2 * NTILE:3 * NTILE, :],
                    in_=stg[NV * bl:NV * (bl + 1), :, :],
                )



# revision 5
# speedup vs baseline: 1.1884x; 1.1884x over previous
"""CWT (GMW filterbank) Trainium2 kernel, v2.

Computes Wx = ifft(Psih * fft(reflect_pad(x)))[..., N1:N1+L] for
x (32, 2048) f32, Psih (256, 4096) f32 -> out (32, 256, 2048) complex64.

Strategy (8 NeuronCores, data-parallel over batch, 4 rows/core):
  - Forward DFT via Cooley-Tukey 4096 = 128 x 32 (DIT): reflect pad is
    materialized by 3 region DMAs + a DVE inner-reversal (the n1-row
    permutation this induces is baked into the stage-1 DFT-128 weights).
    Stage 1 = one [128x128] matmul pair (fp32), twiddle on DVE, one PE
    transpose pair, stage 2 = DFT-32 as a block-diagonal-over-batch
    [128x64] matmul quad (fp32).  ~4 us PE vs ~55 us for the dense DFT,
    and no 16 MB/iteration DFT-matrix HBM traffic.
  - P = Psih (.) xh in fp16 (pre-scaled by 1/4096 via the stage-2
    constants so E can be stored as raw +-1-range phases; avoids fp16
    subnormals).  re on DVE, im on GPSIMD.
  - Banded inverse DFT in fp16 (same PE speed as bf16, 8x the mantissa):
    per-octave k-tile bands at threshold 5e-3 (29 (octave,ktile) pairs).
    Mirror symmetry E[k, 4096-n] = conj(E[k, n]) halves the matmuls:
    U/V/W/Z accumulate over n in [1024, 2048) only; right-half outputs
    assembled with reversed APs.  E tiles persist in SBUF across the
    rep loop (loaded once in the prologue).
  - The 8 concurrent PSUM accumulation groups (U,V,W,Z x 2 n-tiles) map
    1:1 onto the 8 PSUM banks.  start=True clears has_written for the
    whole bank, so interleaved groups MUST be bank-disjoint.
  - Center column n=2048 (self-mirror): per-octave (-1)^p projections of
    P emitted at body end (58 LDW-bound matmuls overlapping the next
    iteration's P-gen via the For_i pipeline), written by 32 tiny
    PSUM->DRAM DMAs.
  - Mirror assembly: V/W copies on ACT (scalar), left combines on DVE,
    right (reversed) combines on GPSIMD; outputs DMA out as
    (b, a, n, 2) f32 in 4x 32-partition chunks to spread HWDGE queues.

Build notes inherited from v1 (hard-won):
  - Use bacc.Bacc() + nc.compile(): plain bass.Bass() fails walrus
    codegen with "Too many sync wait commands".
  - DVE tensor_tensor may read only ONE operand from PSUM.
  - DVE/ACT/GPSIMD are partition-locked; partition permutations need
    DMA or PE transposes (or host-side constant permutation).
  - A single big out-DMA serializes on one HWDGE queue.
"""

import numpy as np

import concourse.bass as bass
import concourse.bacc as bacc
import concourse.mybir as mybir
import concourse.tile as tile
from concourse.bass_utils import run_bass_kernel_spmd

B = 32          # batch
L = 2048        # signal length
UP = 4096       # padded length
N1 = 1024       # left pad (slice offset)
NA = 256        # scales
NV = 32         # voices/octave
NO = 8          # octaves
KF = 2048       # used frequency bins
NC = 8          # cores
BPC = B // NC   # batch rows per core (4)
KT = KF // 128  # k tiles (16)
NTILE = 512     # output columns per matmul
BAND_THRESH = 5e-3

_CACHE = {}


def _host_constants(Psih):
    """FFT stage constants, E filter-bank tiles (fp16), band table."""
    # stage-1 DFT-128 weights with the n1 partition permutation induced by
    # the staged reflect-pad load (regions 1/3 land with n1 reversed)
    n1_of_p = np.arange(128)
    n1_of_p[0:32] = 31 - np.arange(32)
    n1_of_p[96:128] = 223 - np.arange(96, 128)
    th = 2 * np.pi * n1_of_p[:, None] * np.arange(128)[None, :] / 128
    w128_dev = np.stack([np.cos(th), -np.sin(th)], 1).astype(np.float32)

    # twiddle T[k1, n2] = exp(-2i pi k1 n2 / 4096)
    tt = 2 * np.pi * np.arange(128)[:, None] * np.arange(32)[None, :] / UP
    tw_dev = np.stack([np.cos(tt), -np.sin(tt)], 1).astype(np.float32)

    # stage-2 DFT-32 block-diagonal over batch, cols k2 < 16 (k < 2048),
    # scaled by 1/UP so xh (hence P) carries the ifft normalization
    t2 = 2 * np.pi * np.arange(32)[:, None] * np.arange(16)[None, :] / 32
    C32 = np.cos(t2) / UP
    S32 = np.sin(t2) / UP
    c32blk = np.zeros((128, 3, 64), np.float32)
    for b in range(BPC):
        c32blk[32 * b:32 * b + 32, 0, 16 * b:16 * b + 16] = C32
        c32blk[32 * b:32 * b + 32, 1, 16 * b:16 * b + 16] = S32
        c32blk[32 * b:32 * b + 32, 2, 16 * b:16 * b + 16] = -S32

    id128 = np.eye(128, dtype=np.float32)

    # E tiles: raw phases (x UP vs the true inverse-DFT matrix; the 1/UP
    # lives in c32blk): E'[k, n] = exp(2i pi k n / UP), n in [N1, N1+L)
    kk = np.arange(KF)[:, None]
    nn = np.arange(N1, N1 + L // 2)[None, :]
    ph = 2 * np.pi * kk * nn / UP
    Er = np.cos(ph).reshape(KT, 128, 2, NTILE)
    Ei = np.sin(ph).reshape(KT, 128, 2, NTILE)
    e_dev = np.empty((2, KT, 128, 2, NTILE), dtype=np.float16)
    e_dev[:, :, :, 0, :] = Er.transpose(2, 0, 1, 3).astype(np.float16)
    e_dev[:, :, :, 1, :] = Ei.transpose(2, 0, 1, 3).astype(np.float16)

    # center column: E'[k, 2048] = (-1)^k, independent of kt
    epm_dev = ((-1.0) ** (np.arange(128) % 2)).astype(np.float16)[:, None]

    # PsihT device layout: (k_in 128, kt, a 256) f32
    psiht = np.ascontiguousarray(
        Psih[:, :KF].T.reshape(KT, 128, NA).transpose(1, 0, 2)
    ).astype(np.float32)

    bands = []
    for o in range(NO):
        sub = Psih[NV * o:NV * (o + 1), :KF]
        ks = np.nonzero((sub > BAND_THRESH * 2.0).any(axis=0))[0]
        bands.append((int(ks.min()) // 128, int(ks.max()) // 128 + 1))

    return w128_dev, tw_dev, c32blk, id128, e_dev, epm_dev, psiht, bands


def _rev_ap(ap2d, last_col, count):
    """Columns [last_col, last_col-1, ...] of a [128, C] AP."""
    return bass.AP(
        ap2d.tensor,
        ap2d.offset + last_col * ap2d.ap[-1][0],
        [list(ap2d.ap[0]), [-ap2d.ap[-1][0], count]],
    )


def _rev_inner(ap3, n):
    """Reverse the innermost dim (size n) of a 3-dim AP."""
    return bass.AP(
        ap3.tensor, ap3.offset + (n - 1) * ap3.ap[-1][0],
        [list(ap3.ap[0]), list(ap3.ap[1]), [-ap3.ap[-1][0], n]],
    )


def _build_program(w128_dev, tw_dev, c32blk, id128, e_dev, epm_dev, bands,
                   reps=1, variant="full"):
    f32 = mybir.dt.float32
    fp16 = mybir.dt.float16

    nc = bacc.Bacc()
    x_in = nc.dram_tensor("x", [BPC, L], f32, kind="ExternalInput")
    psih_in = nc.dram_tensor("psiht", [128, KT, NA], f32, kind="ExternalInput")
    out_t = nc.dram_tensor("out", [BPC, NA, L, 2], f32, kind="ExternalOutput")

    w128_c = nc.inline_tensor(w128_dev, name="w128c")
    tw_c = nc.inline_tensor(tw_dev, name="twc")
    c32_c = nc.inline_tensor(c32blk, name="c32c")
    id_c = nc.inline_tensor(id128, name="idc")
    e_c = nc.inline_tensor(e_dev, name="econst")
    epm_c = nc.inline_tensor(epm_dev, name="epmc")

    with tile.TileContext(nc) as tc:
        with (
            tc.tile_pool(name="persist", bufs=1) as persist,
            tc.tile_pool(name="pfix", bufs=1) as pfix,
            tc.tile_pool(name="stg", bufs=3) as stgp,
            tc.tile_pool(name="ps_m", bufs=2, space="PSUM") as ps_m,
        ):
            # ---- prologue: inputs + persistent constants ----
            xpt = persist.tile([128, BPC, 32], f32, tag="xp")
            s13 = persist.tile([128, BPC, 32], f32, tag="s13")
            for b in range(BPC):
                nc.sync.dma_start(
                    out=xpt[32:96, b],
                    in_=bass.AP(x_in, L * b, [[32, 64], [1, 32]]),
                )
                nc.sync.dma_start(
                    out=s13[0:32, b],
                    in_=bass.AP(x_in, L * b + 1, [[32, 32], [1, 32]]),
                )
                nc.sync.dma_start(
                    out=s13[96:128, b],
                    in_=bass.AP(x_in, L * b + 1023, [[32, 32], [1, 32]]),
                )
            nc.vector.tensor_copy(out=xpt[0:32], in_=_rev_inner(s13[0:32], 32))
            nc.vector.tensor_copy(out=xpt[96:128], in_=_rev_inner(s13[96:128], 32))

            psih_sb = persist.tile([128, KT, NA], f32, tag="psih")
            nc.sync.dma_start(out=psih_sb, in_=psih_in[:])
            # funnel: absorb the psih DMA wait into DVE's clock
            scratch = persist.tile([1, 4], f32, tag="scratch")
            nc.vector.tensor_copy(out=scratch[0:1, 0:1], in_=psih_sb[0:1, 0, 0:1])

            w128_sb = persist.tile([128, 2, 128], f32, tag="w128")
            nc.sync.dma_start(out=w128_sb, in_=w128_c[:])
            tw_sb = persist.tile([128, 2, 32], f32, tag="tw")
            nc.sync.dma_start(out=tw_sb, in_=tw_c[:])
            c32_sb = persist.tile([128, 3, 64], f32, tag="c32")
            nc.sync.dma_start(out=c32_sb, in_=c32_c[:])
            id_sb = persist.tile([128, 128], f32, tag="id")
            nc.sync.dma_start(out=id_sb, in_=id_c[:])
            epm_sb = persist.tile([128, 1], fp16, tag="epm")
            nc.sync.dma_start(out=epm_sb, in_=epm_c[:])

            etiles = {}
            for lnt in range(2):
                for kt in range(KT):
                    et = persist.tile([128, 2, NTILE], fp16, tag=f"e{lnt}_{kt}")
                    nc.sync.dma_start(out=et, in_=e_c[lnt, kt])
                    etiles[(lnt, kt)] = et

            # dummy transpose absorbs the identity-DMA wait on PE
            dmy = ps_m.tile([128, 2, NTILE], f32, tag="uv", name="dmy")
            nc.tensor.transpose(dmy[:, 0, 0:128], id_sb, id_sb)

            def body():
                _emit_body(
                    nc, bands, out_t, persist, pfix, stgp, ps_m,
                    xpt, psih_sb, w128_sb, tw_sb, c32_sb, id_sb, epm_sb,
                    etiles, f32, fp16, variant,
                )

            if reps == 1:
                body()
            else:
                with tc.For_i(0, reps, 1):
                    body()
    nc.compile()
    return nc


def _emit_body(nc, bands, out_t, persist, pfix, stgp, ps_m,
               xpt, psih_sb, w128_sb, tw_sb, c32_sb, id_sb, epm_sb,
               etiles, f32, fp16, variant):
    skip_fwd = "nofwd" in variant
    skip_out = "noout" in variant
    skip_mm = "nomm" in variant
    skip_ctr = "noctr" in variant

    # ---- forward FFT (one ps_m "uv" slot hosts all fwd psum) ----
    xh_all = persist.tile([128, 2, BPC, KT], f32, tag="xh")
    if not skip_fwd:
        fwd = ps_m.tile([128, 2, NTILE], f32, tag="uv", name="fwd")
        bre = fwd[:, 0, 0:128]
        bim = fwd[:, 1, 0:128]
        xp_flat = xpt.rearrange("p b n -> p (b n)")
        nc.tensor.matmul(bre, w128_sb[:, 0, :], xp_flat, start=True, stop=True)
        nc.tensor.matmul(bim, w128_sb[:, 1, :], xp_flat, start=True, stop=True)

        dd = pfix.tile([128, 2, BPC, 32], f32, tag="dd")
        t1 = pfix.tile([128, BPC, 32], f32, tag="t1")
        t2 = pfix.tile([128, BPC, 32], f32, tag="t2")
        brr = bre.rearrange("p (b n) -> p b n", b=BPC)
        bir = bim.rearrange("p (b n) -> p b n", b=BPC)
        tre = tw_sb[:, 0, None, :].to_broadcast((128, BPC, 32))
        tim = tw_sb[:, 1, None, :].to_broadcast((128, BPC, 32))
        nc.vector.tensor_tensor(t1, brr, tre, mybir.AluOpType.mult)
        nc.vector.tensor_tensor(t2, bir, tim, mybir.AluOpType.mult)
        nc.vector.tensor_sub(dd[:, 0], t1, t2)
        nc.vector.tensor_tensor(t1, brr, tim, mybir.AluOpType.mult)
        nc.vector.tensor_tensor(t2, bir, tre, mybir.AluOpType.mult)
        nc.vector.tensor_add(dd[:, 1], t1, t2)

        dtp_re = fwd[:, 0, 128:256]
        dtp_im = fwd[:, 1, 128:256]
        nc.tensor.transpose(dtp_re, dd[:, 0].rearrange("p b n -> p (b n)"), id_sb)
        nc.tensor.transpose(dtp_im, dd[:, 1].rearrange("p b n -> p (b n)"), id_sb)
        dts = pfix.tile([128, 2, 128], f32, tag="dts")
        nc.scalar.copy(out=dts[:, 0, :], in_=dtp_re)
        nc.scalar.copy(out=dts[:, 1, :], in_=dtp_im)

        xre = fwd[:, 0, 256:320]
        xim = fwd[:, 1, 256:320]
        nc.tensor.matmul(xre, dts[:, 0, :], c32_sb[:, 0, :], start=True, stop=False)
        nc.tensor.matmul(xim, dts[:, 0, :], c32_sb[:, 2, :], start=True, stop=False)
        nc.tensor.matmul(xre, dts[:, 1, :], c32_sb[:, 1, :], start=False, stop=True)
        nc.tensor.matmul(xim, dts[:, 1, :], c32_sb[:, 0, :], start=False, stop=True)
        nc.vector.tensor_copy(
            out=xh_all.rearrange("p c b k -> p c (b k)"),
            in_=fwd[:, :, 256:320],
        )

    # ---- P generation, high kt first (octave 0's band is ready first) ----
    P_re, P_im = {}, {}
    for kt in reversed(range(KT)):
        psih_ap = (
            psih_sb[:, kt, :]
            .rearrange("p (o a) -> p o a", o=NO)[:, :, None, :]
            .to_broadcast((128, NO, BPC, NV))
        )
        for comp, dst, eng in ((0, P_re, nc.vector), (1, P_im, nc.gpsimd)):
            pt = pfix.tile([128, NO * BPC * NV], fp16, tag=f"P{comp}_{kt}")
            if skip_fwd:
                eng.tensor_copy(
                    out=pt.rearrange("p (o b a) -> p o b a", o=NO, b=BPC),
                    in_=psih_ap,
                )
            else:
                xh_ap = (
                    xh_all[:, comp, :, kt][:, None, :, None]
                    .to_broadcast((128, NO, BPC, NV))
                )
                eng.tensor_tensor(
                    pt.rearrange("p (o b a) -> p o b a", o=NO, b=BPC),
                    psih_ap,
                    xh_ap,
                    mybir.AluOpType.mult,
                )
            dst[kt] = pt

    # ---- banded mirror inverse DFT ----
    for o in range(NO):
        if skip_mm:
            continue
        klo, khi = bands[o]
        kts = list(range(klo, khi))
        osl = slice(o * 128, (o + 1) * 128)
        uv0 = ps_m.tile([128, 2, NTILE], f32, tag="uv", name="uv0")
        wz0 = ps_m.tile([128, 2, NTILE], f32, tag="wz", name="wz0")
        uv1 = ps_m.tile([128, 2, NTILE], f32, tag="uv", name="uv1")
        wz1 = ps_m.tile([128, 2, NTILE], f32, tag="wz", name="wz1")
        for j, kt in enumerate(kts):
            st, sp = (j == 0), (j == len(kts) - 1)
            pr = P_re[kt][:, osl]
            pi = P_im[kt][:, osl]
            er0 = etiles[(0, kt)][:, 0, :]
            ei0 = etiles[(0, kt)][:, 1, :]
            er1 = etiles[(1, kt)][:, 0, :]
            ei1 = etiles[(1, kt)][:, 1, :]
            # stationary-reuse ordering: 4 streams per LDW
            nc.tensor.matmul(uv0[:, 0, :], pr, er0, start=st, stop=sp)
            nc.tensor.matmul(uv1[:, 0, :], pr, er1, start=st, stop=sp)
            nc.tensor.matmul(wz0[:, 0, :], pr, ei0, start=st, stop=sp)
            nc.tensor.matmul(wz1[:, 0, :], pr, ei1, start=st, stop=sp)
            nc.tensor.matmul(uv0[:, 1, :], pi, ei0, start=st, stop=sp)
            nc.tensor.matmul(uv1[:, 1, :], pi, ei1, start=st, stop=sp)
            nc.tensor.matmul(wz0[:, 1, :], pi, er0, start=st, stop=sp)
            nc.tensor.matmul(wz1[:, 1, :], pi, er1, start=st, stop=sp)

        # U=uv[:,0], V=uv[:,1], W=wz[:,0], Z=wz[:,1]
        vw0 = stgp.tile([128, 2, NTILE], f32, tag="vw")
        nc.scalar.copy(out=vw0[:, 0, :], in_=uv0[:, 1, :])
        nc.scalar.copy(out=vw0[:, 1, :], in_=wz0[:, 0, :])
        stgL0 = stgp.tile([128, NTILE, 2], f32, tag="stg")
        nc.vector.tensor_sub(stgL0[:, :, 0], uv0[:, 0, :], vw0[:, 0, :])
        nc.vector.tensor_add(stgL0[:, :, 1], wz0[:, 1, :], vw0[:, 1, :])
        _dma_out(nc, out_t, stgL0, o, 0, skip_out)

        vw1 = stgp.tile([128, 2, NTILE], f32, tag="vw")
        nc.scalar.copy(out=vw1[:, 0, :], in_=uv1[:, 1, :])
        nc.scalar.copy(out=vw1[:, 1, :], in_=wz1[:, 0, :])
        stgL1 = stgp.tile([128, NTILE, 2], f32, tag="stg")
        nc.vector.tensor_sub(stgL1[:, :, 0], uv1[:, 0, :], vw1[:, 0, :])
        nc.vector.tensor_add(stgL1[:, :, 1], wz1[:, 1, :], vw1[:, 1, :])
        _dma_out(nc, out_t, stgL1, o, 1, skip_out)

        # right tile 1: n in [2560, 3072) mirrors left tile 0
        stgR1 = stgp.tile([128, NTILE, 2], f32, tag="stg")
        nc.vector.tensor_add(stgR1[:, 1:NTILE, 0],
                             _rev_ap(uv0[:, 0, :], NTILE - 1, NTILE - 1),
                             _rev_ap(vw0[:, 0, :], NTILE - 1, NTILE - 1))
        nc.vector.tensor_sub(stgR1[:, 1:NTILE, 1],
                             _rev_ap(wz0[:, 1, :], NTILE - 1, NTILE - 1),
                             _rev_ap(vw0[:, 1, :], NTILE - 1, NTILE - 1))
        nc.vector.tensor_add(stgR1[:, 0:1, 0], uv1[:, 0, 0:1], vw1[:, 0, 0:1])
        nc.vector.tensor_sub(stgR1[:, 0:1, 1], wz1[:, 1, 0:1], vw1[:, 1, 0:1])
        _dma_out(nc, out_t, stgR1, o, 3, skip_out)

        # right tile 0: n in (2048, 2560) mirrors left tile 1; col 0 (the
        # self-mirrored n=2048 column) is written separately at body end
        stgR0 = stgp.tile([128, NTILE, 2], f32, tag="stg")
        nc.vector.tensor_add(stgR0[:, 1:NTILE, 0],
                             _rev_ap(uv1[:, 0, :], NTILE - 1, NTILE - 1),
                             _rev_ap(vw1[:, 0, :], NTILE - 1, NTILE - 1))
        nc.vector.tensor_sub(stgR0[:, 1:NTILE, 1],
                             _rev_ap(wz1[:, 1, :], NTILE - 1, NTILE - 1),
                             _rev_ap(vw1[:, 1, :], NTILE - 1, NTILE - 1))
        if not skip_out:
            for bl in range(BPC):
                nc.sync.dma_start(
                    out=out_t[bl, NV * o:NV * (o + 1),
                              2 * NTILE + 1:3 * NTILE, :],
                    in_=stgR0[NV * bl:NV * (bl + 1), 1:NTILE, :],
                )

    # ---- center column n=2048 (out col 1024): ctr = sum_k P * (-1)^p ----
    if not (skip_ctr or skip_mm):
        ctr = ps_m.tile([128, 2, NTILE], f32, tag="uv", name="ctr")
        for o in range(NO):
            klo, khi = bands[o]
            osl = slice(o * 128, (o + 1) * 128)
            for j, kt in enumerate(range(klo, khi)):
                st, sp = (j == 0), (j == khi - klo - 1)
                nc.tensor.matmul(ctr[:, 0, o:o + 1], P_re[kt][:, osl],
                                 epm_sb, start=st, stop=sp)
                nc.tensor.matmul(ctr[:, 1, o:o + 1], P_im[kt][:, osl],
                                 epm_sb, start=st, stop=sp)
        ctr_sb = pfix.tile([128, 2, NO], f32, tag="ctrsb")
        nc.vector.tensor_copy(out=ctr_sb, in_=ctr[:, :, 0:NO])
        if not skip_out:
            for o in range(NO):
                for bl in range(BPC):
                    nc.sync.dma_start(
                        out=out_t[bl, NV * o:NV * (o + 1), 2 * NTILE, :],
                        in_=bass.AP(
                            ctr_sb.tensor,
                            ctr_sb.offset + NV * bl * ctr_sb.ap[0][0] + o,
                            [[ctr_sb.ap[0][0], NV], [NO, 2]],
                        ),
                    )


def _dma_out(nc, out_t, stg, o, nt, skip_out):
    if skip_out:
        return
    for bl in range(BPC):
        nc.sync.dma_start(
            out=out_t[bl, NV * o:NV * (o + 1), NTILE * nt:NTILE * (nt + 1), :],
            in_=stg[NV * bl:NV * (bl + 1), :, :],
        )


def _get_program(Psih, reps=1, variant="full"):
    key = f"prog{reps}_{variant}"
    if key not in _CACHE:
        if "consts" not in _CACHE:
            _CACHE["consts"] = _host_constants(np.asarray(Psih))
        w128_dev, tw_dev, c32blk, id128, e_dev, epm_dev, psiht, bands = _CACHE["consts"]
        nc = _build_program(w128_dev, tw_dev, c32blk, id128, e_dev, epm_dev,
                            bands, reps=reps, variant=variant)
        _CACHE[key] = (nc, psiht)
    return _CACHE[key]


def kernel(x, Psih=None, **_unused):
    x = np.ascontiguousarray(np.asarray(x), dtype=np.float32)
    if Psih is None:
        raise ValueError("Psih input required")
    nc, psiht = _get_program(Psih)
    in_maps = [
        {"x": np.ascontiguousarray(x[BPC * c:BPC * (c + 1)]), "psiht": psiht}
        for c in range(NC)
    ]
    res = run_bass_kernel_spmd(nc, in_maps, core_ids=list(range(NC)))
    out = np.concatenate([r["out"] for r in res.results], axis=0)
    return out.view(np.complex64)[..., 0]


def bench(x, Psih, iters=20, reps=1, variant="full"):
    """Run the kernel repeatedly on-device; returns (out_complex, times_ns)."""
    import time
    import jax
    from jax.sharding import Mesh, PartitionSpec
    from jax.experimental.shard_map import shard_map
    from concourse import bass2jax

    x = np.ascontiguousarray(np.asarray(x), dtype=np.float32)
    nc, psiht = _get_program(Psih, reps=reps, variant=variant)
    bass2jax.install_neuronx_cc_hook()

    part_name = nc.partition_id_tensor.name if nc.partition_id_tensor else None
    in_names, out_names, out_avals = [], [], []
    for alloc in nc.m.functions[0].allocations:
        if not isinstance(alloc, mybir.MemoryLocationSet):
            continue
        name = alloc.memorylocations[0].name
        if alloc.kind == "ExternalInput":
            if name != part_name:
                in_names.append(name)
        elif alloc.kind == "ExternalOutput":
            out_names.append(name)
            out_avals.append(
                jax.core.ShapedArray(
                    tuple(alloc.tensor_shape), mybir.dt.np(alloc.dtype)
                )
            )
    n_params = len(in_names)
    all_names = in_names + out_names
    if part_name is not None:
        all_names = all_names + [part_name]

    def _body(*args):
        operands = list(args)
        if part_name is not None:
            operands.append(bass2jax.partition_id_tensor())
        outs = bass2jax._bass_exec_p.bind(
            *operands,
            out_avals=tuple(out_avals),
            in_names=tuple(all_names),
            out_names=tuple(out_names),
            lowering_input_output_aliases=(),
            sim_require_finite=True,
            sim_require_nnan=True,
            nc=nc,
        )
        return tuple(outs)

    devices = jax.devices()[:NC]
    mesh = Mesh(np.asarray(devices), ("core",))
    nin = n_params + len(out_names)
    fn = jax.jit(
        shard_map(
            _body,
            mesh=mesh,
            in_specs=(PartitionSpec("core"),) * nin,
            out_specs=(PartitionSpec("core"),) * len(out_names),
            check_rep=False,
        ),
        keep_unused=True,
    )
    in_map = {"x": x, "psiht": np.concatenate([psiht] * NC, axis=0)}
    concat_in = [in_map[n] for n in in_names]
    concat_zeros = [
        np.zeros((NC * a.shape[0], *a.shape[1:]), a.dtype) for a in out_avals
    ]
    sharding = jax.sharding.NamedSharding(mesh, PartitionSpec("core"))
    args = [jax.device_put(a, sharding) for a in concat_in + concat_zeros]
    out_arrs = jax.block_until_ready(fn(*args))  # compile + first run
    times = []
    for _ in range(iters):
        t0 = time.perf_counter()
        out_arrs = jax.block_until_ready(fn(*args))
        times.append((time.perf_counter() - t0) * 1e9)
    out = np.asarray(out_arrs[0]).reshape(NC, BPC, NA, L, 2).reshape(B, NA, L, 2)
    return out.view(np.complex64)[..., 0], times
